# revision 25
# baseline (speedup 1.0000x reference)
"""GRU-D fused Bass kernel for Trainium2, data-parallel over batch on 8 cores.

Reference model: B=512, T=256, D=256, H=1024
  x_tilde = m*x + (1-m)*xm            (exact for binary m: exp-decay cancels)
  inp = [x_tilde; xm; m]              (middle block folds to a constant bias)
  z = sigmoid(inp @ Wz.T + bz); htil = tanh(inp @ Wh.T + bh)   (r unused)
  h' = (1-z)*h + z*htil; out = sigmoid(h_T @ Wout.T + bout)

Device strategy per core (64 batch rows), v2:
  - One fp8 DoubleRow GEMM over all timesteps: xcat=[x_tilde; m] (K=512 as
    2 DoubleRow pairs of 256) against WT=[Wz'|Wh']*16 (M=2048), columns laid
    out batch-major/time-inner. 8 column blocks of 2048 (8 batch x 256 t).
  - PSUM tiles [128, 2048] (4 banks); one wide ScalarE activation per tile:
    data0 = sigmoid(-(u/16)+(-cz)) = 1-z, htil = tanh(u/16+ch), bf16 out.
  - negd1 = -z*htil: either one DVE scalar_tensor_tensor, or (to balance
    engines) GpSimd tensor_tensor mult + DVE tensor_tensor subtract.
  - tensor_tensor_scan on DVE: h = data0*h - negd1 across the whole 2048-col
    tile; data0's first column of each 256-step segment is zeroed so the
    scan state resets at batch-lane boundaries (h_1 = z_1*htil_1).
  - Last time column per lane extracted via strided AP (GpSimd copy).
  - Head: 8 accumulating [128,1]x[128,64] matmuls + sigmoid.
"""

import numpy as np
import ml_dtypes

import concourse.bass as bass
from concourse import bacc
import concourse.tile as tile
from concourse import mybir
from concourse.bass_utils import run_bass_kernel_spmd

bf16 = ml_dtypes.bfloat16
f8 = ml_dtypes.float8_e4m3
F32 = mybir.dt.float32
BF16 = mybir.dt.bfloat16
F8 = mybir.dt.float8e4

N_CORES = 8
B, T, D, H = 512, 256, 256, 1024
BS = B // N_CORES            # 64 batch rows per core
COLS = BS * T                # 16384 moving columns per core
NBLK = 2048                  # moving columns per block (8 batch x 256 t)
BLKS = COLS // NBLK          # 8 blocks
LPB = NBLK // T              # batch lanes per block (8)
NH = NBLK // 512             # N=512 slices per psum tile (4)
NG = 8                       # h groups of 128 (H = 1024)
SW = 16.0                    # fp8 weight scale

LAST_HW_EXEC_NS = None


def _build_nc():
    nc = bacc.Bacc("TRN2", target_bir_lowering=False)

    xt_d = nc.dram_tensor("xt", [D, COLS], F8, kind="ExternalInput")
    mt_d = nc.dram_tensor("mt", [D, COLS], F8, kind="ExternalInput")
    xt16_d = nc.dram_tensor("xt16", [D, COLS], F8, kind="ExternalInput")
    mt16_d = nc.dram_tensor("mt16", [D, COLS], F8, kind="ExternalInput")
    wt_d = nc.dram_tensor("wt", [128, 2, 2, 2 * H], F8, kind="ExternalInput")
    wtr_d = nc.dram_tensor("wtr", [128, 2, 2, 2 * H], F8, kind="ExternalInput")
    ncz_d = nc.dram_tensor("ncz", [128, NG], F32, kind="ExternalInput")
    ch_d = nc.dram_tensor("ch", [128, NG], F32, kind="ExternalInput")
    wout_d = nc.dram_tensor("wout", [128, NG], BF16, kind="ExternalInput")
    bout_d = nc.dram_tensor("bout", [1, 1], F32, kind="ExternalInput")
    o_d = nc.dram_tensor("o", [1, BS], F32, kind="ExternalOutput")

    AF = mybir.ActivationFunctionType
    OP = mybir.AluOpType
    DR = mybir.MatmulPerfMode.DoubleRow

    with tile.TileContext(nc) as tc:
        with (
            tc.tile_pool(name="consts", bufs=1) as consts,
            tc.tile_pool(name="xcat", bufs=3) as xcat_pool,
            tc.tile_pool(name="gact", bufs=8) as gact_pool,
            tc.tile_pool(name="nd", bufs=3) as nd_pool,
            tc.tile_pool(name="sc", bufs=3) as sc_pool,
            tc.tile_pool(name="psum", bufs=2, space="PSUM") as psum_pool,
        ):
            wt_sb = consts.tile([128, 2, 2, 2 * H], F8)
            wtr_sb = consts.tile([128, 2, 2, 2 * H], F8)
            nc.gpsimd.dma_start(out=wt_sb[:, :, :, 0:H], in_=wt_d[:, :, :, 0:H])
            nc.gpsimd.dma_start(out=wtr_sb[:, :, :, 0:H], in_=wtr_d[:, :, :, 0:H])
            nc.gpsimd.dma_start(out=wt_sb[:, :, :, H:], in_=wt_d[:, :, :, H:])
            nc.gpsimd.dma_start(out=wtr_sb[:, :, :, H:], in_=wtr_d[:, :, :, H:])
            ncz_sb = consts.tile([128, NG], F32)
            nc.sync.dma_start(out=ncz_sb, in_=ncz_d[:, :])
            ch_sb = consts.tile([128, NG], F32)
            nc.sync.dma_start(out=ch_sb, in_=ch_d[:, :])
            wout_sb = consts.tile([128, NG], BF16)
            nc.sync.dma_start(out=wout_sb, in_=wout_d[:, :])
            bout_sb = consts.tile([1, 1], F32)
            nc.sync.dma_start(out=bout_sb, in_=bout_d[:, :])
            hT_sb = consts.tile([128, NG, BS], BF16)

            tile_idx = 0
            sizes = [2048] * 8
            n_tiles = len(sizes) * NG
            starts = [sum(sizes[:i]) for i in range(len(sizes))]
            lane0 = [st // T for st in starts]
            for bi, (cs, nb) in enumerate(zip(starts, sizes)):
                # xcat [p, kp, j, c]: kp0 = x_tilde feats, kp1 = m feats,
                # contraction row k = kp*256 + j*128 + p
                xcat = xcat_pool.tile([128, 2, 2, nb], F8, tag="xcat")
                xcat_r = xcat_pool.tile([128, 2, 2, nb], F8, tag="xcat_r")
                for ci in range(2):
                    half = slice(cs + ci * (nb // 2), cs + (ci + 1) * (nb // 2))
                    dst = slice(ci * (nb // 2), (ci + 1) * (nb // 2))
                    nc.sync.dma_start(
                        out=xcat[:, 0, :, dst],
                        in_=xt_d[:, half].rearrange("(j p) c -> p j c", p=128),
                    )
                    nc.sync.dma_start(
                        out=xcat[:, 1, :, dst],
                        in_=mt_d[:, half].rearrange("(j p) c -> p j c", p=128),
                    )
                    nc.sync.dma_start(
                        out=xcat_r[:, 0, :, dst],
                        in_=xt16_d[:, half].rearrange("(j p) c -> p j c", p=128),
                    )
                    nc.sync.dma_start(
                        out=xcat_r[:, 1, :, dst],
                        in_=mt16_d[:, half].rearrange("(j p) c -> p j c", p=128),
                    )

                for g in range(NG):
                    d0 = gact_pool.tile([128, nb], BF16, tag="d0")
                    ht = gact_pool.tile([128, nb], BF16, tag="ht")
                    for gate in range(2):
                        ps = psum_pool.tile([128, nb], F32, tag="psum")
                        mo = gate * H + g * 128
                        for h4 in range(nb // 512):
                            sl = slice(h4 * 512, (h4 + 1) * 512)
                            for kp in range(2):
                                nc.tensor.matmul(
                                    ps[:, sl],
                                    wt_sb[:, kp, :, mo:mo + 128],
                                    xcat[:, kp, :, sl],
                                    start=(kp == 0), stop=False,
                                    perf_mode=DR,
                                )
                            for kp in range(2):
                                nc.tensor.matmul(
                                    ps[:, sl],
                                    wtr_sb[:, kp, :, mo:mo + 128],
                                    xcat_r[:, kp, :, sl],
                                    start=False, stop=(kp == 1),
                                    perf_mode=DR,
                                )
                        if gate == 0:
                            nc.scalar.activation(
                                d0, ps, AF.Sigmoid,
                                bias=ncz_sb[:, g:g + 1], scale=-1.0 / SW,
                            )
                        else:
                            nc.scalar.activation(
                                ht, ps, AF.Tanh,
                                bias=ch_sb[:, g:g + 1], scale=1.0 / SW,
                            )

                    # negd1 = (d0-1)*ht = -z*htil
                    nd = nd_pool.tile([128, nb], BF16)
                    if tile_idx % 7 == 0 or tile_idx == n_tiles - 1:
                        nc.vector.scalar_tensor_tensor(
                            nd, d0, 1.0, ht, OP.subtract, OP.mult,
                        )
                    else:
                        tmp = nd_pool.tile([128, nb], BF16, tag="tmp")
                        nc.gpsimd.tensor_tensor(tmp, d0, ht, OP.mult)
                        nc.vector.tensor_tensor(nd, tmp, ht, OP.subtract)
                    tile_idx += 1

                    # zero data0 at the first step of every batch lane so the
                    # scan resets: h_1 = 0*state - negd1_1 = z_1*htil_1
                    zap = bass.AP(
                        tensor=d0.tensor, offset=d0.offset,
                        ap=[list(d0.ap[0]), [T, nb // T]],
                    )
                    nc.vector.memset(zap, 0.0)

                    sc = sc_pool.tile([128, nb], BF16)
                    nc.vector.tensor_tensor_scan(
                        sc, d0, nd, 0.0, OP.mult, OP.subtract,
                    )
                    # keep only the last time column of each batch lane
                    src = bass.AP(
                        tensor=sc.tensor, offset=sc.offset + (T - 1),
                        ap=[list(sc.ap[0]), [T, nb // T]],
                    )
                    nc.vector.tensor_copy(
                        hT_sb[:, g, lane0[bi]:lane0[bi] + nb // T], src,
                    )

            psh = psum_pool.tile([1, BS], F32, tag="psum")
            for g in range(NG):
                nc.tensor.matmul(
                    psh, wout_sb[:, g:g + 1], hT_sb[:, g, :],
                    start=(g == 0), stop=(g == NG - 1),
                )
            osb = consts.tile([1, BS], F32)
            nc.scalar.activation(
                osb, psh, AF.Sigmoid, bias=bout_sb[0:1, 0:1], scale=1.0,
            )
            nc.sync.dma_start(out=o_d[:, :], in_=osb)

    nc.compile()
    return nc


_NC = None


def _get_nc():
    global _NC
    if _NC is None:
        _NC = _build_nc()
    return _NC


def prepare_in_maps(X, M, input_means, gamma_x, Wz, bz, Wr, br, Wh, bh,
                    Wout, bout):
    X = np.asarray(X)
    M = np.asarray(M)
    xm64 = np.asarray(input_means, dtype=np.float64)
    xmf = np.asarray(input_means, dtype=np.float32)

    Wz = np.asarray(Wz)
    Wh = np.asarray(Wh)

    # lhsT [K=512, M=2048]: k rows 0:256 x_tilde feats, 256:512 m feats;
    # cols 0:1024 z gate, 1024:2048 h gate. DoubleRow layout [p, kp, j, m]
    # with k = kp*256 + j*128 + p, scaled by SW in fp8.
    WT = np.concatenate(
        [
            np.concatenate([Wz[:, :D].T, Wz[:, 2 * D:].T], axis=0),
            np.concatenate([Wh[:, :D].T, Wh[:, 2 * D:].T], axis=0),
        ],
        axis=1,
    )
    W16 = WT * SW
    wt = np.ascontiguousarray(
        W16.reshape(2, 2, 128, 2 * H).transpose(2, 0, 1, 3)
    ).astype(f8)
    R16 = (W16 - wt.transpose(1, 2, 0, 3).reshape(512, 2 * H)
           .astype(np.float32)) * 16.0
    wtr = np.ascontiguousarray(
        R16.reshape(2, 2, 128, 2 * H).transpose(2, 0, 1, 3)
    ).astype(f8)

    cz = Wz[:, D:2 * D].astype(np.float64) @ xm64 + np.asarray(bz, np.float64)
    ch = Wh[:, D:2 * D].astype(np.float64) @ xm64 + np.asarray(bh, np.float64)
    ncz = np.ascontiguousarray((-cz).reshape(NG, 128).T).astype(np.float32)
    chh = np.ascontiguousarray(ch.reshape(NG, 128).T).astype(np.float32)
    wout = np.ascontiguousarray(
        np.asarray(Wout)[0].reshape(NG, 128).T
    ).astype(bf16)
    boutv = np.asarray(bout, np.float32).reshape(1, 1)

    # host: x_tilde = m*x + (1-m)*xm (exact for binary m), quantize to fp8
    Xt_f = M * X + (1.0 - M) * xmf
    Xt8 = Xt_f.astype(f8)
    Xt16 = (Xt_f * (1.0 / 16.0)).astype(f8)
    M8 = M.astype(f8)
    M16 = (M * (1.0 / 16.0)).astype(f8)

    in_maps = []
    for c in range(N_CORES):
        sl = slice(c * BS, (c + 1) * BS)
        # [d, b, t] -> [D, COLS], columns batch-major time-inner
        xt = np.ascontiguousarray(Xt8[sl].transpose(2, 0, 1)).reshape(D, COLS)
        mt = np.ascontiguousarray(M8[sl].transpose(2, 0, 1)).reshape(D, COLS)
        xt16 = np.ascontiguousarray(
            Xt16[sl].transpose(2, 0, 1)).reshape(D, COLS)
        mt16 = np.ascontiguousarray(
            M16[sl].transpose(2, 0, 1)).reshape(D, COLS)
        in_maps.append(
            {
                "xt": xt, "mt": mt, "xt16": xt16, "mt16": mt16,
                "wt": wt, "wtr": wtr, "ncz": ncz, "ch": chh,
                "wout": wout, "bout": boutv,
            }
        )
    return in_maps


def kernel(X, M, input_means, gamma_x, Wz, bz, Wr, br, Wh, bh, Wout, bout):
    global LAST_HW_EXEC_NS
    in_maps = prepare_in_maps(X, M, input_means, gamma_x, Wz, bz, Wr, br,
                              Wh, bh, Wout, bout)

    res = run_bass_kernel_spmd(_get_nc(), in_maps, list(range(N_CORES)))
    if res.exec_time_ns:
        LAST_HW_EXEC_NS = res.exec_time_ns

    out = np.concatenate(
        [res.results[c]["o"][0] for c in range(N_CORES)]
    ).astype(np.float32)
    return out
